# revision 2
# baseline (speedup 1.0000x reference)
"""CenterLoss kernel for Trainium2 (8 NeuronCores, data-parallel over batch).

reference:  mean(clip(rowsum((x - labels @ centers)^2), 1e-12, 1e12))
labels are exact one-hot rows, so labels @ centers is an embedding gather:
    idx[b]  = max_index(labels[b, :])           (DVE max_index, query = 1.0)
    c[b]    = centers[idx[b], :]                (indirect DMA row gather)
    ps[b]   = rowsum((x[b] - c[b])^2)           (DVE sub -> bf16, ACT square+accum)
Per-core output is the [128, 8] tile of per-sample sums; the host applies
the clip (never binding for this data, but exact) and takes the mean.

Schedule: labels + x share the sync HWDGE ring FIFO (labels first, so the
index chain and the SWDGE gathers start ~10us in and overlap the x stream);
gathers ride the gpsimd SWDGE queue; sub/square pipeline per tile behind
the gathers.
"""

import numpy as np

import concourse.bacc as bacc
import concourse.bass as bass
import concourse.mybir as mybir
from concourse.tile import TileContext
from concourse.bass_utils import run_bass_kernel_spmd

F32 = mybir.dt.float32
BF16 = mybir.dt.bfloat16
U32 = mybir.dt.uint32

NCORES = 8
B = 8192          # full batch
C = 751           # num classes
D = 2048          # feature dim
BS = B // NCORES  # batch per core = 1024
P = 128           # partitions
NT = BS // P      # batch tiles per core = 8

CLIP_LO, CLIP_HI = 1e-12, 1e12


def build_nc():
    nc = bacc.Bacc(
        "TRN2",
        target_bir_lowering=False,
        debug=False,
        num_devices=NCORES,
    )
    x = nc.dram_tensor("x", [BS, D], F32, kind="ExternalInput")
    labels = nc.dram_tensor("labels", [BS, C], F32, kind="ExternalInput")
    centers = nc.dram_tensor("centers", [C, D], F32, kind="ExternalInput")
    out = nc.dram_tensor("out", [P, NT], F32, kind="ExternalOutput")

    with TileContext(nc) as tc:
        with (
            tc.tile_pool(name="big", bufs=1) as bigpool,
            tc.tile_pool(name="dif", bufs=2) as dpool,
            tc.tile_pool(name="dsq", bufs=1) as sqpool,
            tc.tile_pool(name="small", bufs=1) as spool,
        ):
            ones = spool.tile([P, 8], F32)
            idxs = spool.tile([P, NT, 8], U32)
            acc = spool.tile([P, NT], F32)
            lbig = bigpool.tile([P, NT, C], F32)
            xbig = bigpool.tile([P, NT, D], F32)
            ctile = bigpool.tile([P, NT, D], F32)

            nc.vector.memset(ones[:], 1.0)

            labels_r = labels.rearrange("(n p) c -> p n c", p=P)
            x_r = x.rearrange("(n p) d -> p n d", p=P)

            # labels first on the sync ring (FIFO) so idx/gather start early
            with tc.high_priority():
                for g in range(4):
                    nc.sync.dma_start(
                        out=lbig[:, 2 * g:2 * g + 2, :],
                        in_=labels_r[:, 2 * g:2 * g + 2, :],
                    )
            # x behind labels on the same ring: 2 large DMAs
            for g in range(2):
                nc.sync.dma_start(
                    out=xbig[:, 4 * g:4 * g + 4, :],
                    in_=x_r[:, 4 * g:4 * g + 4, :],
                )

            # one-hot -> index (DVE), then row gather (SWDGE)
            for n in range(NT):
                nc.vector.max_index(
                    out=idxs[:, n, :], in_max=ones[:], in_values=lbig[:, n, :]
                )
                nc.gpsimd.indirect_dma_start(
                    out=ctile[:, n, :],
                    out_offset=None,
                    in_=centers[:],
                    in_offset=bass.IndirectOffsetOnAxis(
                        ap=idxs[:, n, 0:1], axis=0
                    ),
                )

            for n in range(NT):
                dif = dpool.tile([P, D], BF16)
                nc.vector.tensor_sub(
                    out=dif[:], in0=xbig[:, n, :], in1=ctile[:, n, :]
                )
                dsq = sqpool.tile([P, D], BF16)
                nc.scalar.activation(
                    out=dsq[:],
                    in_=dif[:],
                    func=mybir.ActivationFunctionType.Square,
                    accum_out=acc[:, n:n + 1],
                )

            nc.sync.dma_start(out=out[:], in_=acc[:])

    nc.compile()
    return nc


_NC = None


def _get_nc():
    global _NC
    if _NC is None:
        _NC = build_nc()
    return _NC


def run_sharded(inputs: dict, trace: bool = False):
    """Shard, run on 8 cores, return (per_sample [B] f32, BassKernelResults)."""
    x = np.ascontiguousarray(np.asarray(inputs["x"], dtype=np.float32))
    labels = np.ascontiguousarray(np.asarray(inputs["labels"], dtype=np.float32))
    centers = np.ascontiguousarray(np.asarray(inputs["centers"], dtype=np.float32))
    assert x.shape == (B, D) and labels.shape == (B, C) and centers.shape == (C, D)

    in_maps = [
        {
            "x": np.ascontiguousarray(x[k * BS:(k + 1) * BS]),
            "labels": np.ascontiguousarray(labels[k * BS:(k + 1) * BS]),
            "centers": centers,
        }
        for k in range(NCORES)
    ]
    res = run_bass_kernel_spmd(
        _get_nc(), in_maps, core_ids=list(range(NCORES)), trace=trace
    )
    # out[p, n] holds sample k*BS + n*P + p
    per_sample = np.concatenate(
        [res.results[k]["out"].T.reshape(-1) for k in range(NCORES)]
    )
    return per_sample, res


def kernel(x, labels, centers):
    per_sample, _ = run_sharded({"x": x, "labels": labels, "centers": centers})
    per_sample = np.clip(per_sample, CLIP_LO, CLIP_HI)
    return np.asarray(per_sample.mean(dtype=np.float64), dtype=np.float32)


# revision 5
# speedup vs baseline: 1.1203x; 1.1203x over previous
"""CenterLoss kernel for Trainium2 (8 NeuronCores, data-parallel over batch).

reference:  mean(clip(rowsum((x - labels @ centers)^2), 1e-12, 1e12))
labels are exact one-hot rows, so labels @ centers is an embedding gather:
    idx[b]  = max_index(labels[b, :])           (DVE max_index, query = 1.0)
    c[b]    = centers[idx[b], :]                (indirect DMA row gather)
    ps[b]   = rowsum((x[b] - c[b])^2)           (DVE sub -> bf16, ACT square+accum)
Per-core output is the [128, 8] tile of per-sample sums; the host applies
the clip (never binding for this data, but exact) and takes the mean.

Schedule: labels + x share the sync HWDGE ring FIFO (labels first, so the
index chain and the SWDGE gathers start ~10us in and overlap the x stream);
gathers ride the gpsimd SWDGE queue; sub/square pipeline per tile behind
the gathers.
"""

import numpy as np

import concourse.bacc as bacc
import concourse.bass as bass
import concourse.mybir as mybir
from concourse.tile import TileContext
from concourse.bass_utils import run_bass_kernel_spmd

F32 = mybir.dt.float32
BF16 = mybir.dt.bfloat16
U32 = mybir.dt.uint32

NCORES = 8
B = 8192          # full batch
C = 751           # num classes
D = 2048          # feature dim
BS = B // NCORES  # batch per core = 1024
P = 128           # partitions
NT = BS // P      # batch tiles per core = 8

CLIP_LO, CLIP_HI = 1e-12, 1e12


def build_nc():
    nc = bacc.Bacc(
        "TRN2",
        target_bir_lowering=False,
        debug=False,
        num_devices=NCORES,
    )
    x = nc.dram_tensor("x", [BS, D], F32, kind="ExternalInput")
    labels = nc.dram_tensor("labels", [BS, C], F32, kind="ExternalInput")
    centers = nc.dram_tensor("centers", [C, D], F32, kind="ExternalInput")
    out = nc.dram_tensor("out", [P, NT], F32, kind="ExternalOutput")

    with TileContext(nc) as tc:
        with (
            tc.tile_pool(name="big", bufs=1) as bigpool,
            tc.tile_pool(name="dif", bufs=2) as dpool,
            tc.tile_pool(name="dsq", bufs=1) as sqpool,
            tc.tile_pool(name="small", bufs=1) as spool,
        ):
            ones = spool.tile([P, 8], F32)
            idxs = spool.tile([P, NT, 8], U32)
            idx2 = spool.tile([P, NT, 8], U32)
            acc = spool.tile([P, NT], F32)
            lbig = bigpool.tile([P, NT, C], F32)
            xbig = bigpool.tile([P, NT, D], F32)
            ctile = bigpool.tile([P, NT, D], F32)

            nc.vector.memset(ones[:], 1.0)

            labels_r = labels.rearrange("(n p) c -> p n c", p=P)
            x_r = x.rearrange("(n p) d -> p n d", p=P)

            # labels first on the sync ring (FIFO) so idx/gather start early
            with tc.high_priority():
                for g in range(4):
                    nc.sync.dma_start(
                        out=lbig[:, 2 * g:2 * g + 2, :],
                        in_=labels_r[:, 2 * g:2 * g + 2, :],
                    )
            # x behind labels on the same ring: 2 large DMAs
            for g in range(2):
                nc.sync.dma_start(
                    out=xbig[:, 4 * g:4 * g + 4, :],
                    in_=x_r[:, 4 * g:4 * g + 4, :],
                )

            # one-hot -> index (DVE)
            for n in range(NT):
                nc.vector.max_index(
                    out=idxs[:, n, :], in_max=ones[:], in_values=lbig[:, n, :]
                )

            # idx2 = (idxs ^ w) ^ w with w read from xbig: exact, and it
            # gates every gather on the last x DMA so the SWDGE gather
            # stream never overlaps the HWDGE x stream (concurrent SWDGE +
            # HWDGE drops aggregate DMA from ~430 to ~320 GB/s).
            w = xbig[:, NT - 1, 0:1].bitcast(U32)
            nc.vector.tensor_scalar(
                out=idx2[:], in0=idxs[:], scalar1=w, scalar2=w,
                op0=mybir.AluOpType.bitwise_xor, op1=mybir.AluOpType.bitwise_xor,
            )

            # row gathers (SWDGE), solo phase after x
            for n in range(NT):
                nc.gpsimd.indirect_dma_start(
                    out=ctile[:, n, :],
                    out_offset=None,
                    in_=centers[:],
                    in_offset=bass.IndirectOffsetOnAxis(
                        ap=idx2[:, n, 0:1], axis=0
                    ),
                )

            for n in range(NT):
                dif = dpool.tile([P, D], BF16)
                nc.vector.tensor_sub(
                    out=dif[:], in0=xbig[:, n, :], in1=ctile[:, n, :]
                )
                dsq = sqpool.tile([P, D], BF16)
                nc.scalar.activation(
                    out=dsq[:],
                    in_=dif[:],
                    func=mybir.ActivationFunctionType.Square,
                    accum_out=acc[:, n:n + 1],
                )

            nc.sync.dma_start(out=out[:], in_=acc[:])

    nc.compile()
    return nc


_NC = None


def _get_nc():
    global _NC
    if _NC is None:
        _NC = build_nc()
    return _NC


def run_sharded(inputs: dict, trace: bool = False):
    """Shard, run on 8 cores, return (per_sample [B] f32, BassKernelResults)."""
    x = np.ascontiguousarray(np.asarray(inputs["x"], dtype=np.float32))
    labels = np.ascontiguousarray(np.asarray(inputs["labels"], dtype=np.float32))
    centers = np.ascontiguousarray(np.asarray(inputs["centers"], dtype=np.float32))
    assert x.shape == (B, D) and labels.shape == (B, C) and centers.shape == (C, D)

    in_maps = [
        {
            "x": np.ascontiguousarray(x[k * BS:(k + 1) * BS]),
            "labels": np.ascontiguousarray(labels[k * BS:(k + 1) * BS]),
            "centers": centers,
        }
        for k in range(NCORES)
    ]
    res = run_bass_kernel_spmd(
        _get_nc(), in_maps, core_ids=list(range(NCORES)), trace=trace
    )
    # out[p, n] holds sample k*BS + n*P + p
    per_sample = np.concatenate(
        [res.results[k]["out"].T.reshape(-1) for k in range(NCORES)]
    )
    return per_sample, res


def kernel(x, labels, centers):
    per_sample, _ = run_sharded({"x": x, "labels": labels, "centers": centers})
    per_sample = np.clip(per_sample, CLIP_LO, CLIP_HI)
    return np.asarray(per_sample.mean(dtype=np.float64), dtype=np.float32)
